# revision 36
# baseline (speedup 1.0000x reference)
"""BatchAlignmentLoss on 8 Trainium2 NeuronCores.

Strategy: shard the feature dim D=2048 across 8 cores (256 cols each).
Each core streams its [8192, 256] slice of the three feature matrices
(the host pre-permutes rows so each 2048-row chunk is one 128-class
block in class-major order, giving the DMA one contiguous 16 KiB run
per partition), computes partial row sq-norms (AllReduce'd per chunk,
24 KiB), normalize-and-casts each row slice to bf16 with 1/norm folded
into the cast, and segment-reduces rows into [512, 256] class sums on
the PE with a constant bf16 identity stationary (bf16 moving streams
1 row/cycle vs fp32's 4 — phase A is DMA-bound, not PE-bound).  The
chunk loop is software-pipelined (chunk c+1's load+squares emit before
chunk c's post-AllReduce casts) so the in-order ACT/DVE streams never
stall on a collective, and the last chunk runs a per-matrix AllReduce
pipeline so only matrix r's chain trails the final DMA.  Each class
block's PSUM bank completes with its chunk and is evacuated (plus
center-norm partials) under the next chunk's stream.  The tail computes
center norms (6 KiB AllReduce), pairwise logits partials (contraction
over the local 256 cols), ReduceScatters the [512,512]x3 logits so each
core log-softmaxes 64 rows per pair, and a final [128,8] AllReduce
combines the intra/inter partial sums into the scalar loss.  A general
fp32 one-hot path covers arbitrary labels.

Algebraic simplifications vs the reference (exact up to fp rounding):
  l2norm(s * inv_cnt) == l2norm(s)           (counts cancel)
  mean||f - c[label]||^2 == 2 - (2/N) * sum_p s_p . c_p
so neither counts nor a per-sample center gather are needed.
"""

import numpy as np

N = 8192
P = 512
D = 2048
NCORES = 8
DL = D // NCORES          # 256 cols per core
TAU = 0.5
NJ = 16                   # N // P occurrences per class (fast path)
NCHUNK = 4                # phase-A chunks (2048 rows each)

_CACHE = {}


def _legalize_waits(nc, mybir):
    """This walrus build accepts at most 1 sync wait per instruction
    (2 on InstEventSemaphore); Tile's scheduler can attach more. Hoist
    the extras onto fresh single-wait nops inserted just before the
    offending instruction (same engine, so ordering is preserved)."""
    for fn in nc.m.functions:
        for bb in fn.blocks:
            insts = bb.instructions
            i = 0
            while i < len(insts):
                inst = insts[i]
                si = getattr(inst, "sync_info", None)
                if si is None:
                    i += 1
                    continue
                waits = list(si.on_wait)
                cap = 2 if isinstance(inst, mybir.InstEventSemaphore) else 1
                if len(waits) <= cap:
                    i += 1
                    continue
                extras, keep = waits[:-cap], waits[-cap:]
                inst.sync_info = mybir.SyncInfo(
                    on_wait=keep, on_update=list(si.on_update))
                for k, w in enumerate(extras):
                    nop = mybir.InstNoOp(
                        name=f"{inst.name}.w{k}",
                        sync_info=mybir.SyncInfo(on_wait=[w], on_update=[]),
                        bass_nofuse=True,
                        engine=inst.engine,
                    )
                    nc.register_instruction(nop, overwrite=True)
                    insts.insert(i, nop)
                    i += 1
                i += 1


DEFAULT_OPTS = {
    # slices whose squares go ACT (rest DVE, slice-granular)
    "act_slices": (0, 1, 3, 5, 6, 8, 10, 11, 13, 15),
    "b_act_m": (1,),        # which m's scale-cast goes to ACT (rest DVE)
    "dma_split": 4,         # DMAs per matrix per chunk
    "split_last": True,     # per-matrix AllReduce pipeline on the last chunk
}


def _build_program(fast, repeat=1, opts=None):
    from concourse import bass, mybir
    from concourse import tile as tile_mod

    opts = {**DEFAULT_OPTS, **(opts or {})}
    act_slices = set(opts["act_slices"])
    b_act_m = set(opts["b_act_m"])
    dma_split = opts["dma_split"]
    split_last = opts["split_last"]

    f32 = mybir.dt.float32
    bf16 = mybir.dt.bfloat16
    Alu = mybir.AluOpType
    Act = mybir.ActivationFunctionType
    Ax = mybir.AxisListType

    nc = bass.Bass()
    fin = {}
    for name in ("fv", "fa", "fr"):
        fin[name] = nc.declare_dram_parameter(name, [N, DL], f32, isOutput=False)
    fmats = [fin["fv"], fin["fa"], fin["fr"]]
    dcol_ext = nc.declare_dram_parameter("dcol", [64, 1], f32, isOutput=False)
    if not fast:
        labm_ext = nc.declare_dram_parameter("labm", [128, 64], f32, isOutput=False)
    loss_ext = nc.declare_dram_parameter("loss", [1, 1], f32, isOutput=True)

    rg = [list(range(NCORES))]

    with tile_mod.TileContext(nc) as tc:
        with (
            tc.tile_pool(name="sb", bufs=2) as sb,
            tc.tile_pool(name="sb1", bufs=1) as sb1,
            tc.tile_pool(name="dram", bufs=2, space="DRAM") as dram,
        ):
            # ---- constants / setup ----
            ones128 = sb1.tile([128, 128], f32, tag="ones128")
            nc.vector.memset(ones128[:], 1.0)
            ident = sb1.tile([128, 128], f32, tag="ident")
            nc.gpsimd.affine_select(
                ident[:], ones128[:], pattern=[[-1, 128]], base=0,
                channel_multiplier=1, compare_op=Alu.is_equal, fill=0.0,
            )
            identb = sb1.tile([128, 128], bf16, tag="identb")
            nc.vector.tensor_copy(identb[:], ident[:])
            dcol = sb1.tile([64, 1], f32, tag="dcol")
            nc.sync.dma_start(dcol[:], dcol_ext[:])
            iota512 = sb1.tile([64, 512], f32, tag="iota512")
            nc.gpsimd.iota(iota512[:], pattern=[[1, 512]], base=0,
                           channel_multiplier=0,
                           allow_small_or_imprecise_dtypes=True)
            dgmask = sb1.tile([64, 512], bf16, tag="dgmask")
            nc.vector.tensor_scalar(dgmask[:], iota512[:], dcol[:], None,
                                    Alu.is_equal)
            wvec = sb1.tile([1, 8], f32, tag="wvec")
            nc.vector.memset(wvec[:, 0:3], -2.0 / N)
            nc.vector.memset(wvec[:, 3:6], -1.0 / P)
            nc.vector.memset(wvec[:, 6:8], 0.0)
            if not fast:
                labm = sb1.tile([128, 64], f32, tag="labm")
                nc.sync.dma_start(labm[:], labm_ext[:])
                iota128 = sb1.tile([128, 128], f32, tag="iota128")
                nc.gpsimd.iota(iota128[:], pattern=[[1, 128]], base=0,
                               channel_multiplier=0,
                               allow_small_or_imprecise_dtypes=True)

            for _rep in range(repeat):
                # ---- phase A: stream + row norms + segment matmuls ----
                # PSUM: one accumulation group per bank. v|a fused as a
                # [128,512] rhs into 4 full banks; r alone in 4 half-banks.
                with tc.tile_pool(name="ps_s", bufs=1, space="PSUM") as ps_s:
                    s_va = [ps_s.tile([128, 512], f32, name=f"sva{q}", tag=f"sva{q}")
                            for q in range(4)]
                    s_r = [ps_s.tile([128, 256], f32, name=f"sr{q}", tag=f"sr{q}")
                           for q in range(4)]

                    # Software-pipelined chunk schedule: emit chunk c+1's
                    # load+squares+AllReduce BEFORE chunk c's scale-casts so
                    # the in-order ACT/DVE streams have a full chunk of
                    # AR-independent work to chew on while chunk c's norm
                    # AllReduce is in flight.
                    st = {}

                    def stage1(c):
                        # -- load: 3 x 2 MiB DMAs into one chunk tile --
                        # Fast path: the host pre-permuted rows so chunk c is
                        # class block c in class-major order — partition p's
                        # 16 instance rows are CONTIGUOUS in HBM (one 16 KiB
                        # descriptor per partition instead of 16 x 1 KiB).
                        if fast:
                            t = sb.tile([128, 16, 768], f32, tag="fch", bufs=3)
                        else:
                            t = sb.tile([128, 16, 768], f32, tag="fch", bufs=3)
                        r0 = 2048 * c
                        for m in range(3):
                            if fast:
                                src_ap = fmats[m][r0:r0 + 2048, :].rearrange(
                                    "(p s) d -> p s d", p=128, s=16)
                                for h in range(dma_split):
                                    s0, s1 = (16 * h // dma_split,
                                              16 * (h + 1) // dma_split)
                                    nc.sync.dma_start(
                                        t[:, s0:s1, 256 * m:256 * m + 256],
                                        src_ap[:, s0:s1, :])
                            else:
                                src_ap = fmats[m][r0:r0 + 2048, :].rearrange(
                                    "(x p) d -> p x d", x=16, p=128)
                                nc.sync.dma_start(
                                    t[:, :, 256 * m:256 * m + 256], src_ap)

                        # slice list: (inner, m) -> [128, 256] view + pack col
                        def views():
                            for xx in range(16):
                                for m in range(3):
                                    col = xx * 3 + m
                                    yield t[:, xx,
                                            256 * m:256 * m + 256], col

                        # -- row sq-norm partials -> sqpack [128, 48] --
                        # Even slices: ACT Square+accum per 256-col view.
                        # Odd slices: DVE does the whole 768-col slice in one
                        # mult + one 3-col reduce ([128,3,256] view), ~30%
                        # cheaper than three mult+reduce pairs.
                        sqpack = sb.tile([128, 48], f32, tag="sqpack")
                        for xx in range(16):
                            if xx in act_slices:
                                for m in range(3):
                                    col = xx * 3 + m
                                    v = t[:, xx, 256 * m:256 * m + 256]
                                    scr = sb.tile([128, 256], f32, tag="scrA", bufs=1)
                                    nc.scalar.activation(
                                        scr[:], v, Act.Square,
                                        accum_out=sqpack[:, col:col + 1])
                            else:
                                v3 = t[:, xx, :].rearrange(
                                    "p (m d) -> p m d", m=3)
                                scr = sb.tile([128, 3, 256], f32, tag="scrV", bufs=1)
                                nc.vector.tensor_tensor(scr[:], v3, v3,
                                                        Alu.mult)
                                nc.vector.tensor_reduce(
                                    sqpack[:, 3 * xx:3 * xx + 3], scr[:],
                                    Ax.X, Alu.add)

                        # -- AllReduce the 24 KiB of partial sq-norms --
                        nin = dram.tile([128, 48], f32, tag="nin")
                        nout = dram.tile([128, 48], f32, tag="nout")
                        nc.gpsimd.dma_start(nin[:], sqpack[:])
                        nc.gpsimd.collective_compute(
                            "AllReduce", Alu.add, replica_groups=rg,
                            ins=[nin.opt()], outs=[nout.opt()])
                        sqg = sb.tile([128, 48], f32, tag="sqg")
                        nc.gpsimd.dma_start(sqg[:], nout[:])
                        st[c] = (t, sqg)

                    def stage2(c):
                        t, sqg = st.pop(c)
                        # -- 1 / max(sqrt(q), eps) --
                        nsr = sb.tile([128, 48], f32, tag="nsr")
                        nc.scalar.activation(nsr[:], sqg[:], Act.Sqrt)
                        nmx = sb.tile([128, 48], f32, tag="nmx")
                        nc.vector.tensor_scalar(nmx[:], nsr[:], 1e-12, None,
                                                Alu.max)
                        rinv = sb.tile([128, 48], f32, tag="rinv")
                        nc.vector.reciprocal(rinv[:], nmx[:])

                        # -- segment accumulate on PE --
                        # Normalize-and-cast each row slice to bf16 (rinv
                        # folded into the cast), then matmul with a constant
                        # bf16 identity stationary: bf16 streams 1 row/cycle
                        # vs fp32's 4.  v|a fused as one [128,512] moving op.
                        if fast:
                            for s in range(16):
                                tb = sb.tile([128, 768], bf16, tag="tb",
                                             bufs=6)
                                for m in range(3):
                                    col = s * 3 + m
                                    src = t[:, s, 256 * m:256 * m + 256]
                                    dst = tb[:, 256 * m:256 * m + 256]
                                    if m in b_act_m:
                                        nc.scalar.activation(
                                            dst, src, Act.Copy,
                                            scale=rinv[:, col:col + 1])
                                    else:
                                        nc.vector.tensor_scalar(
                                            dst, src,
                                            rinv[:, col:col + 1], None,
                                            Alu.mult)
                                nc.tensor.matmul(
                                    s_va[c][:], identb[:], tb[:, 0:512],
                                    start=(s == 0), stop=(s == 15))
                                nc.tensor.matmul(
                                    s_r[c][:], identb[:], tb[:, 512:768],
                                    start=(s == 0), stop=(s == 15))
                        else:
                            for xx in range(16):
                                rt = 16 * c + xx
                                for ps in range(4):
                                    oh = sb.tile([128, 128], f32, tag="oh")
                                    nc.vector.tensor_scalar(
                                        oh[:], iota128[:], labm[:, rt:rt + 1],
                                        float(-128 * ps), Alu.subtract,
                                        Alu.is_equal)
                                    for m in range(3):
                                        col = xx * 3 + m
                                        ohs = sb.tile([128, 128], f32, tag="ohs")
                                        nc.vector.tensor_scalar(
                                            ohs[:], oh[:],
                                            rinv[:, col:col + 1], None, Alu.mult)
                                        if m < 2:
                                            out_ap = s_va[ps][:, 256 * m:256 * m + 256]
                                            nc.tensor.matmul(
                                                out_ap, ohs[:],
                                                t[:, xx, 256 * m:256 * m + 256],
                                                start=(rt == 0 and m == 0),
                                                stop=(rt == 63 and m == 1))
                                        else:
                                            nc.tensor.matmul(
                                                s_r[ps][:], ohs[:],
                                                t[:, xx, 512:768],
                                                start=(rt == 0), stop=(rt == 63))

                    # evacuate block q's PSUM -> SBUF (+ center sq-norm
                    # partials for the tail); in the fast path block c
                    # completes with chunk c, so this overlaps chunk c+1's
                    # stream instead of trailing the whole phase.
                    s_sb = [[sb1.tile([128, 512], bf16, name=f"ssb{m}{h}",
                                      tag=f"ssb{m}{h}")
                             for h in range(2)] for m in range(3)]
                    qpack = sb1.tile([128, 12], f32, tag="qpack")

                    def evac(q):
                        dst = [(0, s_va[q][:, 0:256]), (1, s_va[q][:, 256:512]),
                               (2, s_r[q][:])]
                        for m, src_ap in dst:
                            d_ap = s_sb[m][q // 2][:, 256 * (q % 2):256 * (q % 2) + 256]
                            if (q + m) % 2 == 0:
                                nc.scalar.copy(d_ap, src_ap)
                            else:
                                nc.vector.tensor_copy(d_ap, src_ap)
                        for m in range(3):
                            scr = sb.tile([128, 256], f32, tag="scrA", bufs=1)
                            nc.scalar.activation(
                                scr[:], s_sb[m][q // 2][:, 256 * (q % 2):
                                                        256 * (q % 2) + 256],
                                Act.Square,
                                accum_out=qpack[:, 4 * m + q:4 * m + q + 1])

                    # Last chunk (fast path): per-matrix AllReduce so the
                    # trailing chain after the final DMA is only matrix r's
                    # squares -> AR -> scale-cast -> matmul, not the whole
                    # chunk's.  Matrices v|a finish under r's DMA.
                    def stage1_last(c):
                        t = sb.tile([128, 16, 768], f32, tag="fch", bufs=3)
                        r0 = 2048 * c
                        sqgs = []
                        for m in range(3):
                            src_ap = fmats[m][r0:r0 + 2048, :].rearrange(
                                "(p s) d -> p s d", p=128, s=16)
                            for h in range(dma_split):
                                s0, s1 = (16 * h // dma_split,
                                          16 * (h + 1) // dma_split)
                                nc.sync.dma_start(
                                    t[:, s0:s1, 256 * m:256 * m + 256],
                                    src_ap[:, s0:s1, :])
                            sqp = sb.tile([128, 16], f32, tag=f"sqp{m}")
                            for xx in range(16):
                                v = t[:, xx, 256 * m:256 * m + 256]
                                acc = sqp[:, xx:xx + 1]
                                if xx % 8 < 5:
                                    scr = sb.tile([128, 256], f32, tag="scrA", bufs=1)
                                    nc.scalar.activation(
                                        scr[:], v, Act.Square, accum_out=acc)
                                else:
                                    scr = sb.tile([128, 256], f32, tag="scrV1", bufs=1)
                                    nc.vector.tensor_tensor(scr[:], v, v,
                                                            Alu.mult)
                                    nc.vector.tensor_reduce(acc, scr[:],
                                                            Ax.X, Alu.add)
                            nin = dram.tile([128, 16], f32, tag=f"nin3{m}")
                            nout = dram.tile([128, 16], f32, tag=f"nout3{m}")
                            nc.gpsimd.dma_start(nin[:], sqp[:])
                            nc.gpsimd.collective_compute(
                                "AllReduce", Alu.add, replica_groups=rg,
                                ins=[nin.opt()], outs=[nout.opt()])
                            sqg = sb.tile([128, 16], f32, tag=f"sqg3{m}")
                            nc.gpsimd.dma_start(sqg[:], nout[:])
                            sqgs.append(sqg)
                        st[c] = (t, sqgs)

                    def _rinv16(sqg, m):
                        nsr = sb.tile([128, 16], f32, tag=f"nsr3{m}")
                        nc.scalar.activation(nsr[:], sqg[:], Act.Sqrt)
                        nmx = sb.tile([128, 16], f32, tag=f"nmx3{m}")
                        nc.vector.tensor_scalar(nmx[:], nsr[:], 1e-12, None,
                                                Alu.max)
                        rinv = sb.tile([128, 16], f32, tag=f"rinv3{m}")
                        nc.vector.reciprocal(rinv[:], nmx[:])
                        return rinv

                    def stage2_last(c):
                        t, sqgs = st.pop(c)
                        rv = [_rinv16(sqgs[m], m) for m in range(2)]
                        for s in range(16):
                            tbva = sb.tile([128, 512], bf16, tag="tbva",
                                           bufs=4)
                            for m in range(2):
                                dst = tbva[:, 256 * m:256 * m + 256]
                                src = t[:, s, 256 * m:256 * m + 256]
                                sc = rv[m][:, s:s + 1]
                                if (s + m) % 2 == 0:
                                    nc.vector.tensor_scalar(dst, src, sc,
                                                            None, Alu.mult)
                                else:
                                    nc.scalar.activation(dst, src, Act.Copy,
                                                         scale=sc)
                            nc.tensor.matmul(
                                s_va[c][:], identb[:], tbva[:],
                                start=(s == 0), stop=(s == 15))
                        rv2 = _rinv16(sqgs[2], 2)
                        for s in range(16):
                            tbr = sb.tile([128, 256], bf16, tag="tbr", bufs=4)
                            src = t[:, s, 512:768]
                            sc = rv2[:, s:s + 1]
                            if s % 2 == 0:
                                nc.vector.tensor_scalar(tbr[:], src, sc,
                                                        None, Alu.mult)
                            else:
                                nc.scalar.activation(tbr[:], src, Act.Copy,
                                                     scale=sc)
                            nc.tensor.matmul(
                                s_r[c][:], identb[:], tbr[:],
                                start=(s == 0), stop=(s == 15))

                    use_split = fast and split_last
                    stage1(0)
                    for c in range(NCHUNK):
                        last = c + 1 == NCHUNK - 1
                        if c + 1 < NCHUNK:
                            if last and use_split:
                                pass  # emitted after stage2(c) below
                            else:
                                stage1(c + 1)
                        stage2(c)
                        if last and use_split:
                            stage1_last(c + 1)
                        if fast:
                            evac(c)
                        if last and use_split:
                            stage2_last(c + 1)
                            evac(c + 1)
                            break
                    if not fast:
                        for q in range(4):
                            evac(q)

                def sb_slice(mat, q):
                    return mat[q // 2][:, 256 * (q % 2):256 * (q % 2) + 256]

                # ---- tail ----
                with tc.tile_pool(name="ps_t", bufs=2, space="PSUM") as ps_t, \
                     tc.tile_pool(name="ps_l", bufs=1, space="PSUM") as ps_l, \
                     tc.tile_pool(name="ps_f", bufs=1, space="PSUM") as ps_f:

                    qin = dram.tile([128, 12], f32, tag="qin")
                    qout = dram.tile([128, 12], f32, tag="qout")
                    nc.gpsimd.dma_start(qin[:], qpack[:])
                    nc.gpsimd.collective_compute(
                        "AllReduce", Alu.add, replica_groups=rg,
                        ins=[qin.opt()], outs=[qout.opt()])
                    qg = sb1.tile([128, 12], f32, tag="qg")
                    nc.gpsimd.dma_start(qg[:], qout[:])

                    csqrt = sb1.tile([128, 12], f32, tag="csqrt")
                    nc.scalar.activation(csqrt[:], qg[:], Act.Sqrt)
                    cmx = sb1.tile([128, 12], f32, tag="cmx")
                    nc.vector.tensor_scalar(cmx[:], csqrt[:], 1e-11, None, Alu.max)
                    rc = sb1.tile([128, 12], f32, tag="rc")
                    nc.vector.reciprocal(rc[:], cmx[:])

                    # final pack: cols 0-2 intra dots, 3-5 inter sums
                    finpack = sb1.tile([128, 8], f32, tag="finpack")
                    nc.vector.memset(finpack[:], 0.0)
                    for m in range(3):
                        scr4 = sb.tile([128, 4], f32, tag="scr4")
                        nc.vector.tensor_tensor(
                            scr4[:], qpack[:, 4 * m:4 * m + 4],
                            rc[:, 4 * m:4 * m + 4], Alu.mult)
                        nc.vector.tensor_reduce(
                            finpack[:, m:m + 1], scr4[:], Ax.X, Alu.add)

                    # centers: scale s in place (s is dead after qpack/intra)
                    c_sb = s_sb
                    for m in range(3):
                        for q in range(4):
                            nc.vector.tensor_scalar(
                                sb_slice(c_sb[m], q), sb_slice(s_sb[m], q),
                                rc[:, 4 * m + q:4 * m + q + 1], None, Alu.mult)
                    cT = [sb1.tile([128, 2, 512], bf16, name=f"cT{m}", tag=f"cT{m}") for m in range(3)]
                    for m in range(3):
                        for q in range(4):
                            for kd in range(2):
                                tp = ps_t.tile([128, 128], bf16, tag="tp")
                                blk = c_sb[m][q // 2][:, 256 * (q % 2) + 128 * kd:
                                                      256 * (q % 2) + 128 * kd + 128]
                                nc.tensor.transpose(tp[:], blk, identb[:])
                                d_ap = cT[m][:, kd, 128 * q:128 * q + 128]
                                if (q + kd) % 2 == 0:
                                    nc.vector.tensor_copy(d_ap, tp[:])
                                else:
                                    nc.scalar.copy(d_ap, tp[:])

                    # pairwise logits partials, scaled by 1/TAU, into RS bounce
                    rs_in = dram.tile([NCORES, 3, 64, 512], bf16, tag="rs_in")
                    rs_out = dram.tile([3, 64, 512], bf16, tag="rs_out")
                    pairs = [(0, 1), (0, 2), (1, 2)]
                    for pi, (A, B) in enumerate(pairs):
                        for pt in range(4):
                            lg = ps_l.tile([128, 512], f32, tag=f"lg{pt}")
                            for kd in range(2):
                                nc.tensor.matmul(
                                    lg[:], cT[A][:, kd, 128 * pt:128 * pt + 128],
                                    cT[B][:, kd, :], start=(kd == 0), stop=(kd == 1))
                            lgs = sb.tile([128, 512], bf16, tag="lgs")
                            if pt % 2 == 0:
                                nc.scalar.activation(lgs[:], lg[:], Act.Copy,
                                                     scale=1.0 / TAU)
                            else:
                                nc.vector.tensor_scalar(lgs[:], lg[:], 1.0 / TAU,
                                                        None, Alu.mult)
                            nc.sync.dma_start(rs_in[2 * pt:2 * pt + 2, pi, :, :], lgs[:])
                    nc.gpsimd.collective_compute(
                        "ReduceScatter", Alu.add, replica_groups=rg,
                        ins=[rs_in.opt()], outs=[rs_out.opt()])
                    lgl = sb1.tile([64, 3, 512], bf16, tag="lgl")
                    nc.sync.dma_start(lgl[:], rs_out[:].rearrange("pi p q -> p pi q"))

                    # row log-softmax diag on this core's 64 rows of each pair
                    for pi in range(3):
                        row = lgl[:, pi, :]
                        mxn = sb.tile([64, 1], f32, tag="mxn")
                        nc.vector.tensor_reduce(mxn[:], row, Ax.X, Alu.max,
                                                negate=True)
                        escr = sb.tile([64, 512], f32, tag="escr")
                        se = sb.tile([64, 1], f32, tag="se")
                        nc.scalar.activation(escr[:], row, Act.Exp, bias=mxn[:],
                                             accum_out=se[:])
                        lse = sb.tile([64, 1], f32, tag="lse")
                        nc.scalar.activation(lse[:], se[:], Act.Ln)
                        dscr = sb.tile([64, 512], f32, tag="dscr")
                        dg = sb.tile([64, 1], f32, tag="dgv")
                        nc.vector.tensor_tensor(dscr[:], row, dgmask[:], Alu.mult)
                        nc.vector.tensor_reduce(dg[:], dscr[:], Ax.X, Alu.add)
                        t1 = sb.tile([64, 1], f32, tag="t1")
                        nc.vector.tensor_tensor(t1[:], dg[:], mxn[:], Alu.add)
                        nc.vector.tensor_tensor(
                            finpack[0:64, 3 + pi:4 + pi], t1[:], lse[:], Alu.subtract)

                    # final AllReduce + partition sum + weighted combine
                    fin_i = dram.tile([128, 8], f32, tag="fin_i")
                    fin_o = dram.tile([128, 8], f32, tag="fin_o")
                    nc.gpsimd.dma_start(fin_i[:], finpack[:])
                    nc.gpsimd.collective_compute(
                        "AllReduce", Alu.add, replica_groups=rg,
                        ins=[fin_i.opt()], outs=[fin_o.opt()])
                    fing = sb1.tile([128, 8], f32, tag="fing")
                    nc.gpsimd.dma_start(fing[:], fin_o[:])
                    csum = ps_f.tile([1, 8], f32, tag="csum")
                    nc.tensor.matmul(csum[:], ones128[:, 0:1], fing[:],
                                     start=True, stop=True)
                    fsum = sb1.tile([1, 8], f32, tag="fsum")
                    nc.vector.tensor_copy(fsum[:], csum[:])
                    scr8 = sb1.tile([1, 8], f32, tag="scr8")
                    lsum = sb1.tile([1, 1], f32, tag="lsum")
                    loss = sb1.tile([1, 1], f32, tag="loss")
                    nc.vector.tensor_tensor(scr8[:], fsum[:], wvec[:], Alu.mult)
                    nc.vector.tensor_reduce(lsum[:], scr8[:], Ax.X, Alu.add)
                    nc.vector.tensor_scalar(loss[:], lsum[:], 6.0, None, Alu.add)
                    nc.sync.dma_start(loss_ext[:], loss[:])

    _legalize_waits(nc, mybir)
    return nc


def _get_program(fast, repeat=1, opts=None):
    key = ("prog", fast, repeat,
           tuple(sorted((opts or {}).items())))
    if key not in _CACHE:
        _CACHE[key] = _build_program(fast, repeat, opts)
    return _CACHE[key]


def _make_in_maps(feat_vp, feat_ap, feat_rp, label, fast):
    if fast:
        # Row permutation so chunk c = class block c, class-major: position
        # 2048c + 16p + s <- original row 512s + 128c + p (class 128c+p,
        # instance s).  Gives the DMA one contiguous 16 KiB run per
        # partition instead of 16 x 1 KiB.
        idx = np.arange(N)
        perm = 512 * (idx % 16) + 128 * (idx >> 11) + ((idx >> 4) & 127)
        feat_vp = feat_vp[perm]
        feat_ap = feat_ap[perm]
        feat_rp = feat_rp[perm]
    in_maps = []
    for k in range(NCORES):
        m = {
            "fv": np.ascontiguousarray(feat_vp[:, DL * k:DL * (k + 1)]),
            "fa": np.ascontiguousarray(feat_ap[:, DL * k:DL * (k + 1)]),
            "fr": np.ascontiguousarray(feat_rp[:, DL * k:DL * (k + 1)]),
            "dcol": np.arange(64 * k, 64 * k + 64, dtype=np.float32).reshape(64, 1),
        }
        if not fast:
            m["labm"] = np.ascontiguousarray(
                label.astype(np.float32).reshape(64, 128).T)
        in_maps.append(m)
    return in_maps


def kernel(feat_vp, feat_ap, feat_rp, label, _trace=False):
    from concourse.bass_utils import run_bass_kernel_spmd

    feat_vp = np.asarray(feat_vp, dtype=np.float32)
    feat_ap = np.asarray(feat_ap, dtype=np.float32)
    feat_rp = np.asarray(feat_rp, dtype=np.float32)
    label = np.asarray(label)
    fast = bool((label == (np.arange(N) % P).astype(label.dtype)).all())

    nc = _get_program(fast)
    in_maps = _make_in_maps(feat_vp, feat_ap, feat_rp, label, fast)
    res = run_bass_kernel_spmd(nc, in_maps, list(range(NCORES)), trace=_trace)
    out = np.asarray(res.results[0]["loss"], dtype=np.float32).reshape(())
    if _trace:
        return out, res
    return out



# revision 37
# speedup vs baseline: 1.1811x; 1.1811x over previous
"""BatchAlignmentLoss on 8 Trainium2 NeuronCores.

Strategy: shard the feature dim D=2048 across 8 cores (256 cols each).
Each core streams its [8192, 256] slice of the three feature matrices
(the host pre-permutes rows so each 2048-row chunk is one 128-class
block in class-major order, giving the DMA one contiguous 16 KiB run
per partition), computes partial row sq-norms (AllReduce'd per chunk,
24 KiB), normalize-and-casts each row slice to bf16 with 1/norm folded
into the cast, and segment-reduces rows into [512, 256] class sums on
the PE with a constant bf16 identity stationary (bf16 moving streams
1 row/cycle vs fp32's 4 — phase A is DMA-bound, not PE-bound).  The
chunk loop is software-pipelined (chunk c+1's load+squares emit before
chunk c's post-AllReduce casts) so the in-order ACT/DVE streams never
stall on a collective, and the last chunk runs a per-matrix AllReduce
pipeline so only matrix r's chain trails the final DMA.  Each class
block's PSUM bank completes with its chunk and is evacuated (plus
center-norm partials) under the next chunk's stream.  The tail computes
center norms (6 KiB AllReduce), pairwise logits partials (contraction
over the local 256 cols), ReduceScatters the [512,512]x3 logits so each
core log-softmaxes 64 rows per pair, and a final [128,8] AllReduce
combines the intra/inter partial sums into the scalar loss.  A general
fp32 one-hot path covers arbitrary labels.

Algebraic simplifications vs the reference (exact up to fp rounding):
  l2norm(s * inv_cnt) == l2norm(s)           (counts cancel)
  mean||f - c[label]||^2 == 2 - (2/N) * sum_p s_p . c_p
so neither counts nor a per-sample center gather are needed.
"""

import numpy as np

N = 8192
P = 512
D = 2048
NCORES = 8
DL = D // NCORES          # 256 cols per core
TAU = 0.5
NJ = 16                   # N // P occurrences per class (fast path)
NCHUNK = 4                # phase-A chunks (2048 rows each)

_CACHE = {}


def _legalize_waits(nc, mybir):
    """This walrus build accepts at most 1 sync wait per instruction
    (2 on InstEventSemaphore); Tile's scheduler can attach more. Hoist
    the extras onto fresh single-wait nops inserted just before the
    offending instruction (same engine, so ordering is preserved)."""
    for fn in nc.m.functions:
        for bb in fn.blocks:
            insts = bb.instructions
            i = 0
            while i < len(insts):
                inst = insts[i]
                si = getattr(inst, "sync_info", None)
                if si is None:
                    i += 1
                    continue
                waits = list(si.on_wait)
                cap = 2 if isinstance(inst, mybir.InstEventSemaphore) else 1
                if len(waits) <= cap:
                    i += 1
                    continue
                extras, keep = waits[:-cap], waits[-cap:]
                inst.sync_info = mybir.SyncInfo(
                    on_wait=keep, on_update=list(si.on_update))
                for k, w in enumerate(extras):
                    nop = mybir.InstNoOp(
                        name=f"{inst.name}.w{k}",
                        sync_info=mybir.SyncInfo(on_wait=[w], on_update=[]),
                        bass_nofuse=True,
                        engine=inst.engine,
                    )
                    nc.register_instruction(nop, overwrite=True)
                    insts.insert(i, nop)
                    i += 1
                i += 1


DEFAULT_OPTS = {
    # slices whose squares go ACT (rest DVE, slice-granular)
    "act_slices": (0, 1, 3, 5, 6, 8, 10, 11, 13, 15),
    "b_act_m": (1,),        # which m's scale-cast goes to ACT (rest DVE)
    "dma_split": 4,         # DMAs per matrix per chunk
    "split_last": True,     # per-matrix AllReduce pipeline on the last chunk
}


def _build_program(fast, repeat=1, opts=None):
    from concourse import bass, mybir
    from concourse import tile as tile_mod

    opts = {**DEFAULT_OPTS, **(opts or {})}
    act_slices = set(opts["act_slices"])
    b_act_m = set(opts["b_act_m"])
    dma_split = opts["dma_split"]
    split_last = opts["split_last"]

    f32 = mybir.dt.float32
    bf16 = mybir.dt.bfloat16
    Alu = mybir.AluOpType
    Act = mybir.ActivationFunctionType
    Ax = mybir.AxisListType

    nc = bass.Bass()
    fin = {}
    for name in ("fv", "fa", "fr"):
        fin[name] = nc.declare_dram_parameter(name, [N, DL], f32, isOutput=False)
    fmats = [fin["fv"], fin["fa"], fin["fr"]]
    dcol_ext = nc.declare_dram_parameter("dcol", [64, 1], f32, isOutput=False)
    if not fast:
        labm_ext = nc.declare_dram_parameter("labm", [128, 64], f32, isOutput=False)
    loss_ext = nc.declare_dram_parameter("loss", [1, 1], f32, isOutput=True)

    rg = [list(range(NCORES))]

    with tile_mod.TileContext(nc) as tc:
        with (
            tc.tile_pool(name="sb", bufs=2) as sb,
            tc.tile_pool(name="sb1", bufs=1) as sb1,
            tc.tile_pool(name="dram", bufs=2, space="DRAM") as dram,
        ):
            # ---- constants / setup ----
            ones128 = sb1.tile([128, 128], f32, tag="ones128")
            nc.vector.memset(ones128[:], 1.0)
            ident = sb1.tile([128, 128], f32, tag="ident")
            nc.gpsimd.affine_select(
                ident[:], ones128[:], pattern=[[-1, 128]], base=0,
                channel_multiplier=1, compare_op=Alu.is_equal, fill=0.0,
            )
            identb = sb1.tile([128, 128], bf16, tag="identb")
            nc.vector.tensor_copy(identb[:], ident[:])
            dcol = sb1.tile([64, 1], f32, tag="dcol")
            nc.sync.dma_start(dcol[:], dcol_ext[:])
            iota512 = sb1.tile([64, 512], f32, tag="iota512")
            nc.gpsimd.iota(iota512[:], pattern=[[1, 512]], base=0,
                           channel_multiplier=0,
                           allow_small_or_imprecise_dtypes=True)
            dgmask = sb1.tile([64, 512], bf16, tag="dgmask")
            nc.vector.tensor_scalar(dgmask[:], iota512[:], dcol[:], None,
                                    Alu.is_equal)
            wvec = sb1.tile([1, 8], f32, tag="wvec")
            nc.vector.memset(wvec[:, 0:3], -2.0 / N)
            nc.vector.memset(wvec[:, 3:6], -1.0 / P)
            nc.vector.memset(wvec[:, 6:8], 0.0)
            if not fast:
                labm = sb1.tile([128, 64], f32, tag="labm")
                nc.sync.dma_start(labm[:], labm_ext[:])
                iota128 = sb1.tile([128, 128], f32, tag="iota128")
                nc.gpsimd.iota(iota128[:], pattern=[[1, 128]], base=0,
                               channel_multiplier=0,
                               allow_small_or_imprecise_dtypes=True)

            for _rep in range(repeat):
                # ---- phase A: stream + row norms + segment matmuls ----
                # PSUM: one accumulation group per bank. v|a fused as a
                # [128,512] rhs into 4 full banks; r alone in 4 half-banks.
                with tc.tile_pool(name="ps_s", bufs=1, space="PSUM") as ps_s:
                    s_va = [ps_s.tile([128, 512], f32, name=f"sva{q}", tag=f"sva{q}")
                            for q in range(4)]
                    s_r = [ps_s.tile([128, 256], f32, name=f"sr{q}", tag=f"sr{q}")
                           for q in range(4)]

                    # Software-pipelined chunk schedule: emit chunk c+1's
                    # load+squares+AllReduce BEFORE chunk c's scale-casts so
                    # the in-order ACT/DVE streams have a full chunk of
                    # AR-independent work to chew on while chunk c's norm
                    # AllReduce is in flight.
                    st = {}

                    def stage1(c):
                        # -- load: 3 x 2 MiB DMAs into one chunk tile --
                        # Fast path: the host pre-permuted rows so chunk c is
                        # class block c in class-major order — partition p's
                        # 16 instance rows are CONTIGUOUS in HBM (one 16 KiB
                        # descriptor per partition instead of 16 x 1 KiB).
                        if fast:
                            t = sb.tile([128, 16, 768], f32, tag="fch", bufs=3)
                        else:
                            t = sb.tile([128, 16, 768], f32, tag="fch", bufs=3)
                        r0 = 2048 * c
                        for m in range(3):
                            if fast:
                                src_ap = fmats[m][r0:r0 + 2048, :].rearrange(
                                    "(p s) d -> p s d", p=128, s=16)
                                for h in range(dma_split):
                                    s0, s1 = (16 * h // dma_split,
                                              16 * (h + 1) // dma_split)
                                    nc.sync.dma_start(
                                        t[:, s0:s1, 256 * m:256 * m + 256],
                                        src_ap[:, s0:s1, :])
                            else:
                                src_ap = fmats[m][r0:r0 + 2048, :].rearrange(
                                    "(x p) d -> p x d", x=16, p=128)
                                nc.sync.dma_start(
                                    t[:, :, 256 * m:256 * m + 256], src_ap)

                        # slice list: (inner, m) -> [128, 256] view + pack col
                        def views():
                            for xx in range(16):
                                for m in range(3):
                                    col = xx * 3 + m
                                    yield t[:, xx,
                                            256 * m:256 * m + 256], col

                        # -- row sq-norm partials -> sqpack [128, 48] --
                        # Even slices: ACT Square+accum per 256-col view.
                        # Odd slices: DVE does the whole 768-col slice in one
                        # mult + one 3-col reduce ([128,3,256] view), ~30%
                        # cheaper than three mult+reduce pairs.
                        sqpack = sb.tile([128, 48], f32, tag="sqpack")
                        for xx in range(16):
                            if xx in act_slices:
                                for m in range(3):
                                    col = xx * 3 + m
                                    v = t[:, xx, 256 * m:256 * m + 256]
                                    scr = sb.tile([128, 256], f32, tag="scrA", bufs=1)
                                    nc.scalar.activation(
                                        scr[:], v, Act.Square,
                                        accum_out=sqpack[:, col:col + 1])
                            else:
                                v3 = t[:, xx, :].rearrange(
                                    "p (m d) -> p m d", m=3)
                                scr = sb.tile([128, 3, 256], f32, tag="scrV", bufs=1)
                                nc.vector.tensor_tensor(scr[:], v3, v3,
                                                        Alu.mult)
                                nc.vector.tensor_reduce(
                                    sqpack[:, 3 * xx:3 * xx + 3], scr[:],
                                    Ax.X, Alu.add)

                        # -- AllReduce the 24 KiB of partial sq-norms --
                        nin = dram.tile([128, 48], f32, tag="nin")
                        nout = dram.tile([128, 48], f32, tag="nout")
                        nc.gpsimd.dma_start(nin[:], sqpack[:])
                        nc.gpsimd.collective_compute(
                            "AllReduce", Alu.add, replica_groups=rg,
                            ins=[nin.opt()], outs=[nout.opt()])
                        sqg = sb.tile([128, 48], f32, tag="sqg")
                        nc.gpsimd.dma_start(sqg[:], nout[:])
                        st[c] = (t, sqg)

                    def stage2(c):
                        t, sqg = st.pop(c)
                        # -- 1 / max(sqrt(q), eps) --
                        nsr = sb.tile([128, 48], f32, tag="nsr")
                        nc.scalar.activation(nsr[:], sqg[:], Act.Sqrt)
                        nmx = sb.tile([128, 48], f32, tag="nmx")
                        nc.vector.tensor_scalar(nmx[:], nsr[:], 1e-12, None,
                                                Alu.max)
                        rinv = sb.tile([128, 48], f32, tag="rinv")
                        nc.vector.reciprocal(rinv[:], nmx[:])

                        # -- segment accumulate on PE --
                        # Normalize-and-cast each row slice to bf16 (rinv
                        # folded into the cast), then matmul with a constant
                        # bf16 identity stationary: bf16 streams 1 row/cycle
                        # vs fp32's 4.  v|a fused as one [128,512] moving op.
                        if fast:
                            for s in range(16):
                                tb = sb.tile([128, 768], bf16, tag="tb",
                                             bufs=8)
                                for m in range(3):
                                    col = s * 3 + m
                                    src = t[:, s, 256 * m:256 * m + 256]
                                    dst = tb[:, 256 * m:256 * m + 256]
                                    if m in b_act_m:
                                        nc.scalar.activation(
                                            dst, src, Act.Copy,
                                            scale=rinv[:, col:col + 1])
                                    else:
                                        nc.vector.tensor_scalar(
                                            dst, src,
                                            rinv[:, col:col + 1], None,
                                            Alu.mult)
                                nc.tensor.matmul(
                                    s_va[c][:], identb[:], tb[:, 0:512],
                                    start=(s == 0), stop=(s == 15))
                                nc.tensor.matmul(
                                    s_r[c][:], identb[:], tb[:, 512:768],
                                    start=(s == 0), stop=(s == 15))
                        else:
                            for xx in range(16):
                                rt = 16 * c + xx
                                for ps in range(4):
                                    oh = sb.tile([128, 128], f32, tag="oh")
                                    nc.vector.tensor_scalar(
                                        oh[:], iota128[:], labm[:, rt:rt + 1],
                                        float(-128 * ps), Alu.subtract,
                                        Alu.is_equal)
                                    for m in range(3):
                                        col = xx * 3 + m
                                        ohs = sb.tile([128, 128], f32, tag="ohs")
                                        nc.vector.tensor_scalar(
                                            ohs[:], oh[:],
                                            rinv[:, col:col + 1], None, Alu.mult)
                                        if m < 2:
                                            out_ap = s_va[ps][:, 256 * m:256 * m + 256]
                                            nc.tensor.matmul(
                                                out_ap, ohs[:],
                                                t[:, xx, 256 * m:256 * m + 256],
                                                start=(rt == 0 and m == 0),
                                                stop=(rt == 63 and m == 1))
                                        else:
                                            nc.tensor.matmul(
                                                s_r[ps][:], ohs[:],
                                                t[:, xx, 512:768],
                                                start=(rt == 0), stop=(rt == 63))

                    # evacuate block q's PSUM -> SBUF (+ center sq-norm
                    # partials for the tail); in the fast path block c
                    # completes with chunk c, so this overlaps chunk c+1's
                    # stream instead of trailing the whole phase.
                    s_sb = [[sb1.tile([128, 512], bf16, name=f"ssb{m}{h}",
                                      tag=f"ssb{m}{h}")
                             for h in range(2)] for m in range(3)]
                    qpack = sb1.tile([128, 12], f32, tag="qpack")

                    def evac(q):
                        dst = [(0, s_va[q][:, 0:256]), (1, s_va[q][:, 256:512]),
                               (2, s_r[q][:])]
                        for m, src_ap in dst:
                            d_ap = s_sb[m][q // 2][:, 256 * (q % 2):256 * (q % 2) + 256]
                            if (q + m) % 2 == 0:
                                nc.scalar.copy(d_ap, src_ap)
                            else:
                                nc.vector.tensor_copy(d_ap, src_ap)
                        for m in range(3):
                            scr = sb.tile([128, 256], f32, tag="scrA", bufs=1)
                            nc.scalar.activation(
                                scr[:], s_sb[m][q // 2][:, 256 * (q % 2):
                                                        256 * (q % 2) + 256],
                                Act.Square,
                                accum_out=qpack[:, 4 * m + q:4 * m + q + 1])

                    # Last chunk (fast path): per-matrix AllReduce so the
                    # trailing chain after the final DMA is only matrix r's
                    # squares -> AR -> scale-cast -> matmul, not the whole
                    # chunk's.  Matrices v|a finish under r's DMA.
                    def stage1_last(c):
                        t = sb.tile([128, 16, 768], f32, tag="fch", bufs=3)
                        r0 = 2048 * c
                        sqgs = []
                        for m in range(3):
                            src_ap = fmats[m][r0:r0 + 2048, :].rearrange(
                                "(p s) d -> p s d", p=128, s=16)
                            for h in range(dma_split):
                                s0, s1 = (16 * h // dma_split,
                                          16 * (h + 1) // dma_split)
                                nc.sync.dma_start(
                                    t[:, s0:s1, 256 * m:256 * m + 256],
                                    src_ap[:, s0:s1, :])
                            sqp = sb.tile([128, 16], f32, tag=f"sqp{m}")
                            for xx in range(16):
                                v = t[:, xx, 256 * m:256 * m + 256]
                                acc = sqp[:, xx:xx + 1]
                                if xx % 8 < 5:
                                    scr = sb.tile([128, 256], f32, tag="scrA", bufs=1)
                                    nc.scalar.activation(
                                        scr[:], v, Act.Square, accum_out=acc)
                                else:
                                    scr = sb.tile([128, 256], f32, tag="scrV1", bufs=1)
                                    nc.vector.tensor_tensor(scr[:], v, v,
                                                            Alu.mult)
                                    nc.vector.tensor_reduce(acc, scr[:],
                                                            Ax.X, Alu.add)
                            nin = dram.tile([128, 16], f32, tag=f"nin3{m}")
                            nout = dram.tile([128, 16], f32, tag=f"nout3{m}")
                            nc.gpsimd.dma_start(nin[:], sqp[:])
                            nc.gpsimd.collective_compute(
                                "AllReduce", Alu.add, replica_groups=rg,
                                ins=[nin.opt()], outs=[nout.opt()])
                            sqg = sb.tile([128, 16], f32, tag=f"sqg3{m}")
                            nc.gpsimd.dma_start(sqg[:], nout[:])
                            sqgs.append(sqg)
                        st[c] = (t, sqgs)

                    def _rinv16(sqg, m):
                        nsr = sb.tile([128, 16], f32, tag=f"nsr3{m}")
                        nc.scalar.activation(nsr[:], sqg[:], Act.Sqrt)
                        nmx = sb.tile([128, 16], f32, tag=f"nmx3{m}")
                        nc.vector.tensor_scalar(nmx[:], nsr[:], 1e-12, None,
                                                Alu.max)
                        rinv = sb.tile([128, 16], f32, tag=f"rinv3{m}")
                        nc.vector.reciprocal(rinv[:], nmx[:])
                        return rinv

                    def stage2_last(c):
                        t, sqgs = st.pop(c)
                        rv = [_rinv16(sqgs[m], m) for m in range(2)]
                        for s in range(16):
                            tbva = sb.tile([128, 512], bf16, tag="tbva",
                                           bufs=6)
                            for m in range(2):
                                dst = tbva[:, 256 * m:256 * m + 256]
                                src = t[:, s, 256 * m:256 * m + 256]
                                sc = rv[m][:, s:s + 1]
                                if (s + m) % 2 == 0:
                                    nc.vector.tensor_scalar(dst, src, sc,
                                                            None, Alu.mult)
                                else:
                                    nc.scalar.activation(dst, src, Act.Copy,
                                                         scale=sc)
                            nc.tensor.matmul(
                                s_va[c][:], identb[:], tbva[:],
                                start=(s == 0), stop=(s == 15))
                        rv2 = _rinv16(sqgs[2], 2)
                        for s in range(16):
                            tbr = sb.tile([128, 256], bf16, tag="tbr", bufs=6)
                            src = t[:, s, 512:768]
                            sc = rv2[:, s:s + 1]
                            if s % 2 == 0:
                                nc.vector.tensor_scalar(tbr[:], src, sc,
                                                        None, Alu.mult)
                            else:
                                nc.scalar.activation(tbr[:], src, Act.Copy,
                                                     scale=sc)
                            nc.tensor.matmul(
                                s_r[c][:], identb[:], tbr[:],
                                start=(s == 0), stop=(s == 15))

                    use_split = fast and split_last
                    stage1(0)
                    for c in range(NCHUNK):
                        last = c + 1 == NCHUNK - 1
                        if c + 1 < NCHUNK:
                            if last and use_split:
                                pass  # emitted after stage2(c) below
                            else:
                                stage1(c + 1)
                        stage2(c)
                        if last and use_split:
                            stage1_last(c + 1)
                        if fast:
                            evac(c)
                        if last and use_split:
                            stage2_last(c + 1)
                            evac(c + 1)
                            break
                    if not fast:
                        for q in range(4):
                            evac(q)

                def sb_slice(mat, q):
                    return mat[q // 2][:, 256 * (q % 2):256 * (q % 2) + 256]

                # ---- tail ----
                with tc.tile_pool(name="ps_t", bufs=2, space="PSUM") as ps_t, \
                     tc.tile_pool(name="ps_l", bufs=1, space="PSUM") as ps_l, \
                     tc.tile_pool(name="ps_f", bufs=1, space="PSUM") as ps_f:

                    qin = dram.tile([128, 12], f32, tag="qin")
                    qout = dram.tile([128, 12], f32, tag="qout")
                    nc.gpsimd.dma_start(qin[:], qpack[:])
                    nc.gpsimd.collective_compute(
                        "AllReduce", Alu.add, replica_groups=rg,
                        ins=[qin.opt()], outs=[qout.opt()])
                    qg = sb1.tile([128, 12], f32, tag="qg")
                    nc.gpsimd.dma_start(qg[:], qout[:])

                    csqrt = sb1.tile([128, 12], f32, tag="csqrt")
                    nc.scalar.activation(csqrt[:], qg[:], Act.Sqrt)
                    cmx = sb1.tile([128, 12], f32, tag="cmx")
                    nc.vector.tensor_scalar(cmx[:], csqrt[:], 1e-11, None, Alu.max)
                    rc = sb1.tile([128, 12], f32, tag="rc")
                    nc.vector.reciprocal(rc[:], cmx[:])

                    # final pack: cols 0-2 intra dots, 3-5 inter sums
                    finpack = sb1.tile([128, 8], f32, tag="finpack")
                    nc.vector.memset(finpack[:], 0.0)
                    for m in range(3):
                        scr4 = sb.tile([128, 4], f32, tag="scr4")
                        nc.vector.tensor_tensor(
                            scr4[:], qpack[:, 4 * m:4 * m + 4],
                            rc[:, 4 * m:4 * m + 4], Alu.mult)
                        nc.vector.tensor_reduce(
                            finpack[:, m:m + 1], scr4[:], Ax.X, Alu.add)

                    # centers: scale s in place (s is dead after qpack/intra)
                    c_sb = s_sb
                    for m in range(3):
                        for q in range(4):
                            nc.vector.tensor_scalar(
                                sb_slice(c_sb[m], q), sb_slice(s_sb[m], q),
                                rc[:, 4 * m + q:4 * m + q + 1], None, Alu.mult)
                    cT = [sb1.tile([128, 2, 512], bf16, name=f"cT{m}", tag=f"cT{m}") for m in range(3)]
                    for m in range(3):
                        for q in range(4):
                            for kd in range(2):
                                tp = ps_t.tile([128, 128], bf16, tag="tp")
                                blk = c_sb[m][q // 2][:, 256 * (q % 2) + 128 * kd:
                                                      256 * (q % 2) + 128 * kd + 128]
                                nc.tensor.transpose(tp[:], blk, identb[:])
                                d_ap = cT[m][:, kd, 128 * q:128 * q + 128]
                                if (q + kd) % 2 == 0:
                                    nc.vector.tensor_copy(d_ap, tp[:])
                                else:
                                    nc.scalar.copy(d_ap, tp[:])

                    # pairwise logits partials, scaled by 1/TAU, into RS bounce
                    rs_in = dram.tile([NCORES, 3, 64, 512], bf16, tag="rs_in")
                    rs_out = dram.tile([3, 64, 512], bf16, tag="rs_out")
                    pairs = [(0, 1), (0, 2), (1, 2)]
                    for pi, (A, B) in enumerate(pairs):
                        for pt in range(4):
                            lg = ps_l.tile([128, 512], f32, tag=f"lg{pt}")
                            for kd in range(2):
                                nc.tensor.matmul(
                                    lg[:], cT[A][:, kd, 128 * pt:128 * pt + 128],
                                    cT[B][:, kd, :], start=(kd == 0), stop=(kd == 1))
                            lgs = sb.tile([128, 512], bf16, tag="lgs")
                            if pt % 2 == 0:
                                nc.scalar.activation(lgs[:], lg[:], Act.Copy,
                                                     scale=1.0 / TAU)
                            else:
                                nc.vector.tensor_scalar(lgs[:], lg[:], 1.0 / TAU,
                                                        None, Alu.mult)
                            nc.sync.dma_start(rs_in[2 * pt:2 * pt + 2, pi, :, :], lgs[:])
                    nc.gpsimd.collective_compute(
                        "ReduceScatter", Alu.add, replica_groups=rg,
                        ins=[rs_in.opt()], outs=[rs_out.opt()])
                    lgl = sb1.tile([64, 3, 512], bf16, tag="lgl")
                    nc.sync.dma_start(lgl[:], rs_out[:].rearrange("pi p q -> p pi q"))

                    # row log-softmax diag on this core's 64 rows of each pair
                    for pi in range(3):
                        row = lgl[:, pi, :]
                        mxn = sb.tile([64, 1], f32, tag="mxn")
                        nc.vector.tensor_reduce(mxn[:], row, Ax.X, Alu.max,
                                                negate=True)
                        escr = sb.tile([64, 512], f32, tag="escr")
                        se = sb.tile([64, 1], f32, tag="se")
                        nc.scalar.activation(escr[:], row, Act.Exp, bias=mxn[:],
                                             accum_out=se[:])
                        lse = sb.tile([64, 1], f32, tag="lse")
                        nc.scalar.activation(lse[:], se[:], Act.Ln)
                        dscr = sb.tile([64, 512], f32, tag="dscr")
                        dg = sb.tile([64, 1], f32, tag="dgv")
                        nc.vector.tensor_tensor(dscr[:], row, dgmask[:], Alu.mult)
                        nc.vector.tensor_reduce(dg[:], dscr[:], Ax.X, Alu.add)
                        t1 = sb.tile([64, 1], f32, tag="t1")
                        nc.vector.tensor_tensor(t1[:], dg[:], mxn[:], Alu.add)
                        nc.vector.tensor_tensor(
                            finpack[0:64, 3 + pi:4 + pi], t1[:], lse[:], Alu.subtract)

                    # final AllReduce + partition sum + weighted combine
                    fin_i = dram.tile([128, 8], f32, tag="fin_i")
                    fin_o = dram.tile([128, 8], f32, tag="fin_o")
                    nc.gpsimd.dma_start(fin_i[:], finpack[:])
                    nc.gpsimd.collective_compute(
                        "AllReduce", Alu.add, replica_groups=rg,
                        ins=[fin_i.opt()], outs=[fin_o.opt()])
                    fing = sb1.tile([128, 8], f32, tag="fing")
                    nc.gpsimd.dma_start(fing[:], fin_o[:])
                    csum = ps_f.tile([1, 8], f32, tag="csum")
                    nc.tensor.matmul(csum[:], ones128[:, 0:1], fing[:],
                                     start=True, stop=True)
                    fsum = sb1.tile([1, 8], f32, tag="fsum")
                    nc.vector.tensor_copy(fsum[:], csum[:])
                    scr8 = sb1.tile([1, 8], f32, tag="scr8")
                    lsum = sb1.tile([1, 1], f32, tag="lsum")
                    loss = sb1.tile([1, 1], f32, tag="loss")
                    nc.vector.tensor_tensor(scr8[:], fsum[:], wvec[:], Alu.mult)
                    nc.vector.tensor_reduce(lsum[:], scr8[:], Ax.X, Alu.add)
                    nc.vector.tensor_scalar(loss[:], lsum[:], 6.0, None, Alu.add)
                    nc.sync.dma_start(loss_ext[:], loss[:])

    _legalize_waits(nc, mybir)
    return nc


def _get_program(fast, repeat=1, opts=None):
    key = ("prog", fast, repeat,
           tuple(sorted((opts or {}).items())))
    if key not in _CACHE:
        _CACHE[key] = _build_program(fast, repeat, opts)
    return _CACHE[key]


def _make_in_maps(feat_vp, feat_ap, feat_rp, label, fast):
    if fast:
        # Row permutation so chunk c = class block c, class-major: position
        # 2048c + 16p + s <- original row 512s + 128c + p (class 128c+p,
        # instance s).  Gives the DMA one contiguous 16 KiB run per
        # partition instead of 16 x 1 KiB.
        idx = np.arange(N)
        perm = 512 * (idx % 16) + 128 * (idx >> 11) + ((idx >> 4) & 127)
        feat_vp = feat_vp[perm]
        feat_ap = feat_ap[perm]
        feat_rp = feat_rp[perm]
    in_maps = []
    for k in range(NCORES):
        m = {
            "fv": np.ascontiguousarray(feat_vp[:, DL * k:DL * (k + 1)]),
            "fa": np.ascontiguousarray(feat_ap[:, DL * k:DL * (k + 1)]),
            "fr": np.ascontiguousarray(feat_rp[:, DL * k:DL * (k + 1)]),
            "dcol": np.arange(64 * k, 64 * k + 64, dtype=np.float32).reshape(64, 1),
        }
        if not fast:
            m["labm"] = np.ascontiguousarray(
                label.astype(np.float32).reshape(64, 128).T)
        in_maps.append(m)
    return in_maps


def kernel(feat_vp, feat_ap, feat_rp, label, _trace=False):
    from concourse.bass_utils import run_bass_kernel_spmd

    feat_vp = np.asarray(feat_vp, dtype=np.float32)
    feat_ap = np.asarray(feat_ap, dtype=np.float32)
    feat_rp = np.asarray(feat_rp, dtype=np.float32)
    label = np.asarray(label)
    fast = bool((label == (np.arange(N) % P).astype(label.dtype)).all())

    nc = _get_program(fast)
    in_maps = _make_in_maps(feat_vp, feat_ap, feat_rp, label, fast)
    res = run_bass_kernel_spmd(nc, in_maps, list(range(NCORES)), trace=_trace)
    out = np.asarray(res.results[0]["loss"], dtype=np.float32).reshape(())
    if _trace:
        return out, res
    return out

